# revision 1
# baseline (speedup 1.0000x reference)
import sys

if "/opt/trn_rl_repo" not in sys.path:
    sys.path.insert(0, "/opt/trn_rl_repo")

from contextlib import ExitStack

import numpy as np

import concourse.mybir as mybir
from concourse import bacc
from concourse.bass_utils import run_bass_kernel_spmd
from concourse.masks import make_identity
from concourse.tile import TileContext

F32 = mybir.dt.float32
F32R = mybir.dt.float32r

B, T, C, H, D = 8, 512, 1024, 16, 64
MAX_POS = 512
TOPK = 32
P = 128
OT = C // P  # 8 channel tiles
TT = T // P  # 4 token tiles
N_CORES = 8

NEG_BIG = -1e30


def build_program():
    nc = bacc.Bacc(None, target_bir_lowering=False)

    xT_d = nc.declare_dram_parameter("xT", [P, OT, T], F32, isOutput=False)
    wq_d = nc.declare_dram_parameter("wq", [P, OT, C], F32, isOutput=False)
    wk_d = nc.declare_dram_parameter("wk", [P, OT, C], F32, isOutput=False)
    wv_d = nc.declare_dram_parameter("wv", [P, OT, C], F32, isOutput=False)
    wo_d = nc.declare_dram_parameter("wo", [P, OT, C], F32, isOutput=False)
    bqp_d = nc.declare_dram_parameter("bqp", [P, OT], F32, isOutput=False)
    bkp_d = nc.declare_dram_parameter("bkp", [P, OT], F32, isOutput=False)
    bvb_d = nc.declare_dram_parameter("bvb", [P, C], F32, isOutput=False)
    bob_d = nc.declare_dram_parameter("bob", [P, C], F32, isOutput=False)
    gates_d = nc.declare_dram_parameter("gates", [P, H], F32, isOutput=False)
    posb_d = nc.declare_dram_parameter("posb", [H, TT, P, T], F32, isOutput=False)
    out_d = nc.declare_dram_parameter("out", [T, C], F32, isOutput=True)

    Exp = mybir.ActivationFunctionType.Exp
    Copy = mybir.ActivationFunctionType.Copy
    add = mybir.AluOpType.add
    mult = mybir.AluOpType.mult

    with TileContext(nc) as tc, ExitStack() as ctx:
        const = ctx.enter_context(tc.tile_pool(name="const", bufs=1))
        wpool = ctx.enter_context(tc.tile_pool(name="wpool", bufs=2))
        xpool = ctx.enter_context(tc.tile_pool(name="xpool", bufs=1))
        proj = ctx.enter_context(tc.tile_pool(name="proj", bufs=1))
        spool4 = ctx.enter_context(tc.tile_pool(name="spool4", bufs=4))
        spool2 = ctx.enter_context(tc.tile_pool(name="spool2", bufs=2))
        spool3 = ctx.enter_context(tc.tile_pool(name="spool3", bufs=3))
        ppool = ctx.enter_context(tc.tile_pool(name="ppool", bufs=4))
        small = ctx.enter_context(tc.tile_pool(name="small", bufs=6))
        headp = ctx.enter_context(tc.tile_pool(name="headp", bufs=2))
        biasp = ctx.enter_context(tc.tile_pool(name="biasp", bufs=2))
        outp = ctx.enter_context(tc.tile_pool(name="outp", bufs=1))
        psA = ctx.enter_context(tc.tile_pool(name="psA", bufs=1, space="PSUM"))
        psS = ctx.enter_context(tc.tile_pool(name="psS", bufs=3, space="PSUM"))
        psT = ctx.enter_context(tc.tile_pool(name="psT", bufs=2, space="PSUM"))
        psO = ctx.enter_context(tc.tile_pool(name="psO", bufs=2, space="PSUM"))

        ident_f = const.tile([P, P], F32)
        make_identity(nc, ident_f)
        ident_r = const.tile([P, P], F32R)
        nc.vector.tensor_copy(ident_r[:], ident_f[:])
        gates_sb = const.tile([P, H], F32)
        nc.sync.dma_start(gates_sb[:], gates_d[:])
        bqp_sb = const.tile([P, OT], F32)
        nc.sync.dma_start(bqp_sb[:], bqp_d[:])
        bkp_sb = const.tile([P, OT], F32)
        nc.sync.dma_start(bkp_sb[:], bkp_d[:])
        bvb_sb = const.tile([P, C], F32)
        nc.sync.dma_start(bvb_sb[:], bvb_d[:])
        bob_sb = const.tile([P, C], F32)
        nc.sync.dma_start(bob_sb[:], bob_d[:])

        # ---- V projection first (f32r; no selection sensitivity) ----
        wv_sb = wpool.tile([P, OT, C], F32R, tag="w")
        nc.gpsimd.dma_start(wv_sb[:], wv_d[:].bitcast(F32R))
        xR_sb = xpool.tile([P, OT, T], F32R, tag="x")
        nc.gpsimd.dma_start(xR_sb[:], xT_d[:].bitcast(F32R))
        wq_sb = wpool.tile([P, OT, C], F32, tag="w")
        nc.sync.dma_start(wq_sb[:], wq_d[:])
        V_sb = proj.tile([P, TT, C], F32R, tag="v")
        for tt in range(TT):
            for oh in range(2):
                ps = psA.tile([P, T], F32, tag="psA")
                for kt in range(OT):
                    nc.tensor.matmul(
                        ps[:],
                        lhsT=xR_sb[:, kt, tt * P:(tt + 1) * P],
                        rhs=wv_sb[:, kt, oh * 512:(oh + 1) * 512],
                        start=(kt == 0),
                        stop=(kt == OT - 1),
                    )
                nc.vector.tensor_tensor(
                    V_sb[:, tt, oh * 512:(oh + 1) * 512], ps[:],
                    bvb_sb[:, oh * 512:(oh + 1) * 512], op=add,
                )

        # ---- Q/K projections; results split into bf16 hi/lo so the score
        # matmuls can run as 4 exact bf16 terms instead of slow fp32.
        xT_sb = xpool.tile([P, OT, T], F32, tag="x")
        nc.sync.dma_start(xT_sb[:], xT_d[:])
        wk_sb = wpool.tile([P, OT, C], F32, tag="w")
        nc.sync.dma_start(wk_sb[:], wk_d[:])
        BF16 = mybir.dt.bfloat16
        sub_op = mybir.AluOpType.subtract
        Qhi = proj.tile([P, OT, T], BF16, tag="qhi")
        Qlo = proj.tile([P, OT, T], BF16, tag="qlo")
        Khi = proj.tile([P, OT, T], BF16, tag="khi")
        Klo = proj.tile([P, OT, T], BF16, tag="klo")
        for ot in range(OT):
            for w_sb, bias_sb, hi, lo in ((wq_sb, bqp_sb, Qhi, Qlo),
                                          (wk_sb, bkp_sb, Khi, Klo)):
                ps = psA.tile([P, T], F32, tag="psA")
                for kt in range(OT):
                    nc.tensor.matmul(
                        ps[:],
                        lhsT=w_sb[:, kt, ot * P:(ot + 1) * P],
                        rhs=xT_sb[:, kt, :],
                        start=(kt == 0),
                        stop=(kt == OT - 1),
                    )
                nc.vector.tensor_scalar_add(hi[:, ot, :], ps[:], bias_sb[:, ot:ot + 1])
                nc.vector.scalar_tensor_tensor(
                    out=lo[:, ot, :], in0=ps[:], scalar=bias_sb[:, ot:ot + 1],
                    in1=hi[:, ot, :], op0=add, op1=sub_op)

        wo_sb = wpool.tile([P, OT, C], F32R, tag="w")
        nc.gpsimd.dma_start(wo_sb[:], wo_d[:].bitcast(F32R))

        # ---- attention, head pair g = (2g, 2g+1) ----
        AO_sb = proj.tile([P, OT, T], F32R, tag="ao")
        for g in range(OT):
            for hh in range(2):
                h = 2 * g + hh
                prow = 64 * hh
                # f32r matmuls reject output base-partition 64, so each head
                # accumulates into its own base-0 [64, T] bank and odd heads
                # are DMA-shifted into the upper half of AO_sb.
                ao_ps = psO.tile([64, T], F32, tag="psO")
                p_tiles = []
                sums_h = headp.tile([P, TT], F32, tag="sums")
                for it in range(TT):
                    s_ps = psS.tile([P, T], F32, tag="psS")
                    terms = ((Qhi, Khi), (Qhi, Klo), (Qlo, Khi), (Qlo, Klo))
                    for ti, (qq, kk) in enumerate(terms):
                        nc.tensor.matmul(
                            s_ps[:],
                            lhsT=qq[prow:prow + 64, g, it * P:(it + 1) * P],
                            rhs=kk[prow:prow + 64, g, :],
                            start=(ti == 0),
                            stop=(ti == 3),
                        )
                    pb_sb = biasp.tile([P, T], F32, tag="pb")
                    nc.sync.dma_start(pb_sb[:], posb_d[h, it])
                    S_sb = spool4.tile([P, T], F32, tag="S")
                    nc.vector.tensor_tensor(S_sb[:], s_ps[:], pb_sb[:], op=add)

                    # top-32 extraction: 4 rounds of max8 + match_replace.
                    # m_all collects the 32 removed values; S4 = scores with
                    # the top-32 replaced by NEG_BIG.
                    m_all = small.tile([P, 4 * 8], F32, tag="mall")
                    sc0 = spool3.tile([P, T], F32, tag="sc")
                    sc1 = spool3.tile([P, T], F32, tag="sc")
                    sc2 = spool3.tile([P, T], F32, tag="sc")
                    S4 = spool3.tile([P, T], F32, tag="sc")
                    src = S_sb
                    for r, dst4 in enumerate((sc0, sc1, sc2, S4)):
                        nc.vector.max(out=m_all[:, r * 8:(r + 1) * 8], in_=src[:])
                        nc.vector.match_replace(
                            out=dst4[:], in_to_replace=m_all[:, r * 8:(r + 1) * 8],
                            in_values=src[:], imm_value=NEG_BIG)
                        src = dst4
                    # normalizer: sum of kept weights = sum(exp(top-32 values))
                    scrapM = small.tile([P, 4 * 8], F32, tag="scrapM")
                    nc.scalar.activation(scrapM[:], m_all[:], Exp,
                                         accum_out=sums_h[:, it:it + 1])
                    # unnormalized masked weights: exp(S) - exp(S4) is nonzero
                    # exactly at the top-32 positions (bitwise cancellation).
                    E = spool2.tile([P, T], F32, tag="E")
                    nc.scalar.activation(E[:], S_sb[:], Exp)
                    E4 = spool2.tile([P, T], F32, tag="E4")
                    nc.scalar.activation(E4[:], S4[:], Exp)
                    p_u = ppool.tile([P, T], F32, tag="P")
                    nc.gpsimd.tensor_sub(p_u[:], E[:], E4[:])
                    p_tiles.append(p_u)

                # per-head batched normalizer scale = gate / sum
                inv4 = headp.tile([P, TT], F32, tag="inv4")
                nc.vector.reciprocal(inv4[:], sums_h[:])
                scl4 = headp.tile([P, TT], F32, tag="scl4")
                nc.vector.tensor_scalar(scl4[:], inv4[:], gates_sb[:, h:h + 1],
                                        None, op0=mult)
                p_r = []
                for it in range(TT):
                    pr = ppool.tile([P, T], F32R, tag="Pr")
                    nc.scalar.activation(pr[:], p_tiles[it][:], Copy,
                                         scale=scl4[:, it:it + 1])
                    p_r.append(pr)

                # transpose P and accumulate attn_out^T
                for jt in range(TT):
                    pt_ps = psT.tile([P, T], F32R, tag="psT")
                    for it in range(TT):
                        nc.tensor.transpose(
                            pt_ps[:, it * P:(it + 1) * P],
                            p_r[it][:, jt * P:(jt + 1) * P],
                            ident_r[:],
                        )
                    PT_sb = spool2.tile([P, T], F32R, tag="PT")
                    nc.scalar.copy(PT_sb[:], pt_ps[:])
                    nc.tensor.matmul(
                        ao_ps[:],
                        lhsT=V_sb[:, jt, h * 64:(h + 1) * 64],
                        rhs=PT_sb[:],
                        start=(jt == 0),
                        stop=(jt == TT - 1),
                    )
                if hh == 0:
                    nc.scalar.copy(AO_sb[0:64, g, :], ao_ps[:])
                else:
                    stage = spool2.tile([64, T], F32R, tag="stg")
                    nc.scalar.copy(stage[:], ao_ps[:])
                    nc.sync.dma_start(AO_sb[64:128, g, :], stage[:])

        # ---- output projection (f32r) ----
        for tt in range(TT):
            for oh in range(2):
                ps = psA.tile([P, T], F32, tag="psA")
                for ct in range(OT):
                    nc.tensor.matmul(
                        ps[:],
                        lhsT=AO_sb[:, ct, tt * P:(tt + 1) * P],
                        rhs=wo_sb[:, ct, oh * 512:(oh + 1) * 512],
                        start=(ct == 0),
                        stop=(ct == OT - 1),
                    )
                o_sb = outp.tile([P, T], F32, tag="o")
                nc.vector.tensor_tensor(o_sb[:], ps[:], bob_sb[:, oh * 512:(oh + 1) * 512],
                                        op=add)
                nc.sync.dma_start(out_d[tt * P:(tt + 1) * P, oh * 512:(oh + 1) * 512], o_sb[:])

    nc.compile()
    if not nc.is_finalized():
        nc.finalize()
    return nc


def prep_inputs(x, Wq, bq, Wk, bk, Wv, bv, Wo, bo, head_gates, rel_bias):
    """Host-side reshapes/transposes into the layouts the device program wants."""
    x = np.asarray(x, np.float32)
    scale = np.float32(1.0 / np.sqrt(D))

    def to_kpart(w):
        # [C_in, C_out] -> [P, OT, C_out] with c_in = kt*P + p
        return np.ascontiguousarray(
            np.asarray(w, np.float32).reshape(OT, P, C).transpose(1, 0, 2))

    wq_r = to_kpart(np.asarray(Wq, np.float32).T * scale)
    wk_r = to_kpart(np.asarray(Wk, np.float32).T)
    wv_r = to_kpart(np.asarray(Wv, np.float32).T)
    wo_r = to_kpart(np.asarray(Wo, np.float32).T)

    bqp = np.ascontiguousarray((np.asarray(bq, np.float32) * scale).reshape(OT, P).T)
    bkp = np.ascontiguousarray(np.asarray(bk, np.float32).reshape(OT, P).T)
    bvb = np.ascontiguousarray(np.tile(np.asarray(bv, np.float32)[None, :], (P, 1)))
    bob = np.ascontiguousarray(np.tile(np.asarray(bo, np.float32)[None, :], (P, 1)))
    gates = np.ascontiguousarray(
        np.tile(np.asarray(head_gates, np.float32)[None, :], (P, 1)))

    idx = np.arange(T)
    rel = idx[None, :] - idx[:, None] + (MAX_POS - 1)          # [T, T]
    pb = np.asarray(rel_bias, np.float32)[rel]                 # [T, T, H]
    posb = np.ascontiguousarray(
        pb.transpose(2, 0, 1).reshape(H, TT, P, T))            # [H, TT, P, T]

    shared = dict(wq=wq_r, wk=wk_r, wv=wv_r, wo=wo_r, bqp=bqp, bkp=bkp,
                  bvb=bvb, bob=bob, gates=gates, posb=posb)

    in_maps = []
    for b in range(B):
        xT = np.ascontiguousarray(
            x[b].T.reshape(OT, P, T).transpose(1, 0, 2))       # [P, OT, T]
        in_maps.append(dict(xT=xT, **shared))
    return in_maps


_NC_CACHE = {}


def get_program():
    if "nc" not in _NC_CACHE:
        _NC_CACHE["nc"] = build_program()
    return _NC_CACHE["nc"]


def kernel(x, Wq, bq, Wk, bk, Wv, bv, Wo, bo, head_gates, rel_bias):
    nc = get_program()
    in_maps = prep_inputs(x, Wq, bq, Wk, bk, Wv, bv, Wo, bo, head_gates, rel_bias)
    res = run_bass_kernel_spmd(nc, in_maps, list(range(N_CORES)))
    return np.stack([res.results[b]["out"] for b in range(B)], axis=0)

